# revision 13
# baseline (speedup 1.0000x reference)
"""Banded gaussian-masked attention (sparse_attention) on 8 TRN2 NeuronCores.

Math (per batch b, data-parallel over 8 cores):
  Qhat[c,n] = q[c,n] / ||q[c,:]||         (per-channel scale)
  Khat[c,m] = k[c,m] / ||k[:,m]||         (per-key-column scale)
  E[n,m]    = sum_c Qhat[c,n] Khat[c,m]
  A[n,m]    = exp(E[n,m] * gau[n,m])      (gau zero outside a +-114 diagonal band
                                           -> A == 1 there)
  D[m]      = sum_n A[n,m]   ;   attention[n,m] = A[n,m] / D[m]
  out[v,m]  = sum_n v[v,n] attention[n,m]
            = (Vtot[v] + sum_n v[v,n](A[n,m]-1)) / D[m]

Only the diagonal band (25 row-blocks of 128 x 356-wide key windows) is ever
computed on the TensorEngine; everything outside the band is exp(0)=1 and is
produced by broadcast-filling 1/D[m].

Matmul-facing tensors are bf16 (|E*gau| <= 0.06, so exp stays in [0.94, 1.06]
and bf16 rounding is far inside the 2e-2 gate); accumulation stays f32.
"""

import os

import numpy as np

N = 3136          # 56*56
NPAD = 3200       # N padded to 25*128 (query axis only)
NB = 25           # row blocks of 128 query rows (last block: 64 valid rows)
HALF = 114        # max |n-m| with nonzero gau: 2*56 + 2
W = 356           # key-window width per row block: 128 + 2*HALF fits in 356
CV = 21           # value channels
B = 8             # batch == number of cores


def _block_geom(b):
    r0 = 128 * b
    rows = min(128, N - r0)
    w0 = min(max(r0 - HALF, 0), N - W)
    return r0, rows, w0


_NC = None


def _ensure_profiling_shims():
    """Best-effort: make trace=True work under axon in this container.

    The image's ``antenv`` lacks ``axon_hooks`` (the glue module
    ``concourse.bass_utils`` imports when tracing under axon), and
    ``upload_artifacts`` needs a cloud bucket this container doesn't have.
    Install a minimal in-process stand-in for the former (wiring the ctypes
    NTFF hook from ``trn_agent_boot``) and a local-fallback wrapper for the
    latter. No-ops when the real infrastructure is present. Without tracing
    requested (no ``trace=True`` / ``BASS_TRACE``), none of this is used.
    """
    import sys
    import types

    try:
        import antenv.axon_hooks  # noqa: F401
    except ImportError:
        try:
            import antenv

            mod = types.ModuleType("antenv.axon_hooks")
            _state = {"hook": None}
            mod.set_axon_ntff_profile_hook = lambda h: _state.__setitem__("hook", h)
            mod.get_axon_ntff_profile_hook = lambda: _state["hook"]
            sys.modules["antenv.axon_hooks"] = mod
            antenv.axon_hooks = mod
            try:
                from trn_agent_boot.trn_boot import _ntff_profile_via_ctypes

                mod.set_axon_ntff_profile_hook(
                    _ntff_profile_via_ctypes("/opt/axon/libaxon_pjrt.so")
                )
            except Exception:
                pass
        except Exception:
            pass

    try:
        import concourse.bass_utils as bu

        if not getattr(bu.upload_artifacts, "_local_fallback", False):
            _orig_upload = bu.upload_artifacts

            def _upload(tmpdir):
                try:
                    return _orig_upload(tmpdir)
                except Exception:
                    return str(tmpdir)

            _upload._local_fallback = True
            bu.upload_artifacts = _upload
    except Exception:
        pass


def _build():
    import concourse.bacc as bacc
    import concourse.mybir as mybir
    import concourse.tile as tile

    F32 = mybir.dt.float32
    BF16 = mybir.dt.bfloat16
    AF = mybir.ActivationFunctionType

    nc = bacc.Bacc("TRN2", target_bir_lowering=False, debug=False, num_devices=B)

    qd = nc.dram_tensor("qp", [128, 2, NPAD], BF16, kind="ExternalInput").ap()
    kd = nc.dram_tensor("kp", [128, 2, N], BF16, kind="ExternalInput").ap()
    vd = nc.dram_tensor("vp", [CV, N], F32, kind="ExternalInput").ap()
    vtd = nc.dram_tensor("vt", [128, NB, CV + 1], BF16, kind="ExternalInput").ap()
    gsd = nc.dram_tensor("gs", [NB, 128, W], BF16, kind="ExternalInput").ap()
    seld = nc.dram_tensor("sel", [128, 128], F32, kind="ExternalInput").ap()
    attnd = nc.dram_tensor("attn", [N, N], F32, kind="ExternalOutput").ap()
    outvd = nc.dram_tensor("outv", [CV, N], F32, kind="ExternalOutput").ap()

    with tile.TileContext(nc) as tc:
        with (
            tc.tile_pool(name="persist", bufs=1) as persist,
            tc.tile_pool(name="big", bufs=2) as bigpool,
            tc.tile_pool(name="gpool", bufs=3) as gpool,
            tc.tile_pool(name="egpool", bufs=3) as egpool,
            tc.tile_pool(name="ampool", bufs=3) as ampool,
            tc.tile_pool(name="bandpool", bufs=3) as bandpool,
            tc.tile_pool(name="scr", bufs=2) as scr,
            tc.tile_pool(name="ps_e", bufs=2, space="PSUM") as ps_e,
            tc.tile_pool(name="ps_c", bufs=2, space="PSUM") as ps_c,
            tc.tile_pool(name="ps_n", bufs=2, space="PSUM") as ps_n,
        ):
            q_sb = persist.tile([128, 2, NPAD], BF16, tag="q")
            k_sb = persist.tile([128, 2, N], BF16, tag="k")
            v_sb = persist.tile([CV, N], F32, tag="v")
            vt_sb = persist.tile([128, NB, CV + 1], BF16, tag="vt")
            acc = persist.tile([128, N], F32, tag="acc")
            a_st = persist.tile([128, NB, W], BF16, tag="ast")
            ones128 = persist.tile([128, 128], BF16, tag="ones")
            e21 = persist.tile([128, 128], F32, tag="e21")
            qn2p = persist.tile([128, 2, 7], F32, tag="qn2p")
            qn = persist.tile([128, 2], F32, tag="qn")
            vtot = persist.tile([CV, 1], F32, tag="vtot")
            outsb = persist.tile([CV, N], F32, tag="outsb")
            nconst = persist.tile([128, 1], F32, tag="nconst")
            nc.vector.memset(nconst, float(N))

            # split big input DMAs so several queues run them concurrently
            for cc in range(2):
                nc.sync.dma_start(out=k_sb[:, cc, :], in_=kd[:, cc, :])
            for cc in range(2):
                nc.sync.dma_start(out=q_sb[:, cc, :], in_=qd[:, cc, :])
            nc.sync.dma_start(out=v_sb, in_=vd)
            nc.sync.dma_start(out=vt_sb, in_=vtd)
            nc.sync.dma_start(out=e21, in_=seld)

            nc.vector.memset(acc, 0.0)
            nc.vector.memset(ones128, 1.0)

            # ---- k column norms: rkn[m] = 1/sqrt(sum_c k[c,m]^2), broadcast
            # over 128 partitions via a ones-matmul (PE reduces partitions).
            ks2 = bigpool.tile([128, N], BF16, tag="bigb")
            ks2b = bigpool.tile([128, N], BF16, tag="bigb")
            knb = bigpool.tile([128, N], F32, tag="big")
            rkn = bigpool.tile([128, N], F32, tag="big")
            nc.vector.tensor_mul(ks2, k_sb[:, 0, :], k_sb[:, 0, :])
            nc.vector.tensor_mul(ks2b, k_sb[:, 1, :], k_sb[:, 1, :])
            for i in range(7):
                n0 = 512 * i
                cw = min(512, N - n0)
                knp = ps_n.tile([128, 512], F32, tag="psn")
                nc.tensor.matmul(
                    knp[:, :cw], ones128, ks2[:, n0 : n0 + cw], start=True, stop=False
                )
                nc.tensor.matmul(
                    knp[:, :cw], ones128, ks2b[:, n0 : n0 + cw], start=False, stop=True
                )
                nc.scalar.activation(knb[:, n0 : n0 + cw], knp[:, :cw], AF.Sqrt)
            nc.vector.reciprocal_approx_fast(rkn, knb)
            nc.vector.tensor_mul(k_sb[:, 0, :], k_sb[:, 0, :], rkn)
            nc.vector.tensor_mul(k_sb[:, 1, :], k_sb[:, 1, :], rkn)

            # ---- q row norms: 1/sqrt(sum_n q[c,n]^2), then scale q in place.
            for cc in range(2):
                for i in range(7):  # NPAD = 6*512 + 128
                    n0 = 512 * i
                    cw = min(512, NPAD - n0)
                    sq = scr.tile([128, 512], BF16, tag="sq")
                    nc.scalar.activation(
                        sq[:, :cw],
                        q_sb[:, cc, n0 : n0 + cw],
                        AF.Square,
                        accum_out=qn2p[:, cc, i : i + 1],
                    )
            nc.vector.reduce_sum(qn, qn2p, axis=mybir.AxisListType.X)
            nc.scalar.activation(qn, qn, AF.Sqrt)
            nc.vector.reciprocal(qn, qn)
            for cc in range(2):
                nc.vector.tensor_scalar_mul(
                    q_sb[:, cc, :], q_sb[:, cc, :], qn[:, cc : cc + 1]
                )

            nc.vector.reduce_sum(vtot, v_sb, axis=mybir.AxisListType.X)

            invd = persist.tile([128, N], F32, tag="invd")

            # D[m] is final once every covering row-block has accumulated:
            # after block b, columns < F(b) = 128*b + 14 are final (block b+1's
            # window starts exactly there). So 1/D is produced in prefix
            # chunks inside the loop, and each row-block's left-fill and band
            # are written as soon as the columns they need are final. Only
            # the right-fills (columns past the band) wait for the end.
            def F(b):
                return N if b >= NB - 1 else min(128 * b + 14, N)

            def emit_invd_chunk(b):
                lo, hi = (0 if b == 0 else F(b - 1)), F(b)
                if hi <= lo:
                    return
                for n0 in range(lo, hi, 512):
                    cw = min(512, hi - n0)
                    dps = ps_n.tile([128, 512], F32, tag="psn")
                    nc.tensor.matmul(
                        dps[:, :cw], e21, acc[:, n0 : n0 + cw], start=True, stop=True
                    )
                    nc.scalar.activation(
                        dps[:, :cw], dps[:, :cw], AF.Identity, bias=nconst
                    )
                    nc.vector.reciprocal_approx_fast(
                        invd[:, n0 : n0 + cw], dps[:, :cw]
                    )

            def emit_left_fill(rb):
                r0, rows, w0 = _block_geom(rb)
                if w0 > 0:
                    nc.sync.dma_start(
                        out=attnd[r0 : r0 + rows, 0:w0], in_=invd[:rows, 0:w0]
                    )

            def emit_band(rb):
                r0, rows, w0 = _block_geom(rb)
                w1 = w0 + W
                band = bandpool.tile([128, W], F32, tag="band")
                nc.vector.tensor_mul(
                    band[:rows], a_st[:rows, rb, :], invd[:rows, w0:w1]
                )
                nc.sync.dma_start(
                    out=attnd[r0 : r0 + rows, w0:w1], in_=band[:rows]
                )

            def emit_right_fill(rb):
                r0, rows, w0 = _block_geom(rb)
                w1 = w0 + W
                if w1 < N:
                    nc.sync.dma_start(
                        out=attnd[r0 : r0 + rows, w1:N], in_=invd[:rows, w1:N]
                    )

            # earliest loop iteration whose F() covers each block's needs
            band_at = {}
            left_at = {}
            for rb in range(NB):
                r0, rows, w0 = _block_geom(rb)
                w1 = w0 + W
                bb = next(b for b in range(NB) if F(b) >= w1)
                band_at.setdefault(bb, []).append(rb)
                if w0 > 0:
                    lb = next(b for b in range(NB) if F(b) >= w0)
                    left_at.setdefault(lb, []).append(rb)

            # ---- main loop: banded energies, exp, (A-1) contraction with v,
            # incremental 1/D, and interleaved attention writes.
            # vt's column CV is 1.0 on valid rows, so c_ps row CV is the
            # column-sum of (A-1) -> D[m] = N + acc[CV, m].
            for b in range(NB):
                r0, rows, w0 = _block_geom(b)
                g_sb = gpool.tile([128, W], BF16, tag="g")
                nc.sync.dma_start(out=g_sb, in_=gsd[b])
                e_ps = ps_e.tile([128, W], F32, tag="pse")
                nc.tensor.matmul(
                    e_ps,
                    q_sb[:, 0, r0 : r0 + 128],
                    k_sb[:, 0, w0 : w0 + W],
                    start=True,
                    stop=False,
                )
                nc.tensor.matmul(
                    e_ps,
                    q_sb[:, 1, r0 : r0 + 128],
                    k_sb[:, 1, w0 : w0 + W],
                    start=False,
                    stop=True,
                )
                eg = egpool.tile([128, W], F32, tag="eg")
                nc.vector.tensor_mul(eg, e_ps, g_sb)
                nc.scalar.activation(a_st[:, b, :], eg, AF.Exp)
                am = ampool.tile([128, W], BF16, tag="am")
                nc.vector.tensor_scalar_add(am, a_st[:, b, :], -1.0)
                c_ps = ps_c.tile([CV + 1, W], F32, tag="psc")
                nc.tensor.matmul(c_ps, vt_sb[:, b, :], am, start=True, stop=True)
                nc.vector.tensor_add(
                    acc[0 : CV + 1, w0 : w0 + W], acc[0 : CV + 1, w0 : w0 + W], c_ps
                )
                emit_invd_chunk(b)
                for rb in left_at.get(b, []):
                    emit_left_fill(rb)
                for rb in band_at.get(b, []):
                    emit_band(rb)

            # ---- tail: right fills + out_v (need the full 1/D).
            for rb in range(NB):
                emit_right_fill(rb)
            nc.vector.tensor_scalar_add(outsb, acc[0:CV, :], vtot)
            nc.vector.tensor_mul(outsb, outsb, invd[0:CV, :])
            nc.sync.dma_start(out=outvd, in_=outsb)

    nc.compile()
    return nc


def _get_nc():
    global _NC
    if _NC is None:
        _NC = _build()
    return _NC


LAST_RESULT = None


def kernel(q, k, v, gau_kernel):
    _ensure_profiling_shims()
    import ml_dtypes

    from concourse.bass_utils import run_bass_kernel_spmd

    global LAST_RESULT

    BF = ml_dtypes.bfloat16
    q = np.asarray(q, dtype=np.float32)
    k = np.asarray(k, dtype=np.float32)
    v = np.asarray(v, dtype=np.float32)
    gau_kernel = np.asarray(gau_kernel, dtype=np.float32)

    # gau band strips, shared by all cores; pad rows of the last block are 0
    # so their exp is 1 and they contribute nothing.
    gs = np.zeros((NB, 128, W), BF)
    for b in range(NB):
        r0, rows, w0 = _block_geom(b)
        gs[b, :rows, :] = gau_kernel[r0 : r0 + rows, w0 : w0 + W].astype(BF)

    # selector: lhsT with partition CV all-ones -> matmul broadcasts acc[CV,:]
    sel = np.zeros((128, 128), np.float32)
    sel[CV, :] = 1.0

    in_maps = []
    for i in range(B):
        qb = q[i].reshape(256, N)
        qp = np.zeros((128, 2, NPAD), BF)
        qp[:, :, :N] = qb.reshape(2, 128, N).transpose(1, 0, 2).astype(BF)
        kb = k[i].reshape(256, N)
        kp = np.ascontiguousarray(kb.reshape(2, 128, N).transpose(1, 0, 2)).astype(BF)
        vb = np.ascontiguousarray(v[i].reshape(CV, N))
        vt = np.zeros((128, NB, CV + 1), BF)
        for b in range(NB):
            r0, rows, w0 = _block_geom(b)
            vt[:rows, b, :CV] = vb[:, r0 : r0 + rows].T.astype(BF)
            vt[:rows, b, CV] = 1.0
        in_maps.append(
            {"qp": qp, "kp": kp, "vp": vb, "vt": vt, "gs": gs, "sel": sel}
        )

    nc = _get_nc()
    res = run_bass_kernel_spmd(nc, in_maps, core_ids=list(range(B)))
    LAST_RESULT = res
    outs = res.results
    attn = np.stack([np.asarray(outs[i]["attn"], dtype=np.float32) for i in range(B)])
    outv = np.stack(
        [np.asarray(outs[i]["outv"], dtype=np.float32) for i in range(B)]
    ).reshape(B, CV, 56, 56)
    return outv, attn


# revision 17
# speedup vs baseline: 1.0452x; 1.0452x over previous
"""Banded gaussian-masked attention (sparse_attention) on 8 TRN2 NeuronCores.

Math (per batch b, data-parallel over 8 cores):
  Qhat[c,n] = q[c,n] / ||q[c,:]||         (per-channel scale)
  Khat[c,m] = k[c,m] / ||k[:,m]||         (per-key-column scale)
  E[n,m]    = sum_c Qhat[c,n] Khat[c,m]
  A[n,m]    = exp(E[n,m] * gau[n,m])      (gau zero outside a +-114 diagonal band
                                           -> A == 1 there)
  D[m]      = sum_n A[n,m]   ;   attention[n,m] = A[n,m] / D[m]
  out[v,m]  = sum_n v[v,n] attention[n,m]
            = (Vtot[v] + sum_n v[v,n](A[n,m]-1)) / D[m]

Only the diagonal band (25 row-blocks of 128 x 356-wide key windows) is ever
computed on the TensorEngine; everything outside the band is exp(0)=1 and is
produced by broadcast-filling 1/D[m].

Matmul-facing tensors are bf16 (|E*gau| <= 0.06, so exp stays in [0.94, 1.06]
and bf16 rounding is far inside the 2e-2 gate); accumulation stays f32.
"""

import os

import numpy as np

N = 3136          # 56*56
NPAD = 3200       # N padded to 25*128 (query axis only)
NB = 25           # row blocks of 128 query rows (last block: 64 valid rows)
HALF = 114        # max |n-m| with nonzero gau: 2*56 + 2
W = 356           # key-window width per row block: 128 + 2*HALF fits in 356
CV = 21           # value channels
B = 8             # batch == number of cores


def _block_geom(b):
    r0 = 128 * b
    rows = min(128, N - r0)
    w0 = min(max(r0 - HALF, 0), N - W)
    return r0, rows, w0


_NC = None


def _ensure_profiling_shims():
    """Best-effort: make trace=True work under axon in this container.

    The image's ``antenv`` lacks ``axon_hooks`` (the glue module
    ``concourse.bass_utils`` imports when tracing under axon), and
    ``upload_artifacts`` needs a cloud bucket this container doesn't have.
    Install a minimal in-process stand-in for the former (wiring the ctypes
    NTFF hook from ``trn_agent_boot``) and a local-fallback wrapper for the
    latter. No-ops when the real infrastructure is present. Without tracing
    requested (no ``trace=True`` / ``BASS_TRACE``), none of this is used.
    """
    import sys
    import types

    try:
        import antenv.axon_hooks  # noqa: F401
    except ImportError:
        try:
            import antenv

            mod = types.ModuleType("antenv.axon_hooks")
            _state = {"hook": None}
            mod.set_axon_ntff_profile_hook = lambda h: _state.__setitem__("hook", h)
            mod.get_axon_ntff_profile_hook = lambda: _state["hook"]
            sys.modules["antenv.axon_hooks"] = mod
            antenv.axon_hooks = mod
            try:
                from trn_agent_boot.trn_boot import _ntff_profile_via_ctypes

                mod.set_axon_ntff_profile_hook(
                    _ntff_profile_via_ctypes("/opt/axon/libaxon_pjrt.so")
                )
            except Exception:
                pass
        except Exception:
            pass

    try:
        import concourse.bass_utils as bu

        if not getattr(bu.upload_artifacts, "_local_fallback", False):
            _orig_upload = bu.upload_artifacts

            def _upload(tmpdir):
                try:
                    return _orig_upload(tmpdir)
                except Exception:
                    return str(tmpdir)

            _upload._local_fallback = True
            bu.upload_artifacts = _upload
    except Exception:
        pass


def _build():
    import concourse.bacc as bacc
    import concourse.mybir as mybir
    import concourse.tile as tile

    F32 = mybir.dt.float32
    BF16 = mybir.dt.bfloat16
    AF = mybir.ActivationFunctionType

    nc = bacc.Bacc("TRN2", target_bir_lowering=False, debug=False, num_devices=B)

    qd = nc.dram_tensor("qp", [128, 2, NPAD], BF16, kind="ExternalInput").ap()
    kd = nc.dram_tensor("kp", [128, 2, N], BF16, kind="ExternalInput").ap()
    vd = nc.dram_tensor("vp", [CV, N], F32, kind="ExternalInput").ap()
    vtd = nc.dram_tensor("vt", [128, NB, CV + 1], BF16, kind="ExternalInput").ap()
    gsd = nc.dram_tensor("gs", [NB, 128, W], BF16, kind="ExternalInput").ap()
    seld = nc.dram_tensor("sel", [128, 128], F32, kind="ExternalInput").ap()
    attnd = nc.dram_tensor("attn", [N, N], F32, kind="ExternalOutput").ap()
    outvd = nc.dram_tensor("outv", [CV, N], F32, kind="ExternalOutput").ap()

    with tile.TileContext(nc) as tc:
        with (
            tc.tile_pool(name="persist", bufs=1) as persist,
            tc.tile_pool(name="big", bufs=2) as bigpool,
            tc.tile_pool(name="gpool", bufs=4) as gpool,
            tc.tile_pool(name="egpool", bufs=4) as egpool,
            tc.tile_pool(name="ampool", bufs=4) as ampool,
            tc.tile_pool(name="bandpool", bufs=4) as bandpool,
            tc.tile_pool(name="scr", bufs=2) as scr,
            tc.tile_pool(name="ps_e", bufs=3, space="PSUM") as ps_e,
            tc.tile_pool(name="ps_c", bufs=2, space="PSUM") as ps_c,
            tc.tile_pool(name="ps_n", bufs=2, space="PSUM") as ps_n,
        ):
            q_sb = persist.tile([128, 2, NPAD], BF16, tag="q")
            k_sb = persist.tile([128, 2, N], BF16, tag="k")
            v_sb = persist.tile([CV, N], F32, tag="v")
            vt_sb = persist.tile([128, NB, CV + 1], BF16, tag="vt")
            acc = persist.tile([128, N], F32, tag="acc")
            a_st = persist.tile([128, NB, W], BF16, tag="ast")
            ones128 = persist.tile([128, 128], BF16, tag="ones")
            e21 = persist.tile([128, 128], F32, tag="e21")
            qn2p = persist.tile([128, 2, 7], F32, tag="qn2p")
            qn = persist.tile([128, 2], F32, tag="qn")
            vtot = persist.tile([CV, 1], F32, tag="vtot")
            outsb = persist.tile([CV, N], F32, tag="outsb")
            nconst = persist.tile([128, 1], F32, tag="nconst")
            nc.vector.memset(nconst, float(N))

            # split big input DMAs so several queues run them concurrently
            for cc in range(2):
                nc.sync.dma_start(out=k_sb[:, cc, :], in_=kd[:, cc, :])
            for cc in range(2):
                nc.sync.dma_start(out=q_sb[:, cc, :], in_=qd[:, cc, :])
            nc.sync.dma_start(out=v_sb, in_=vd)
            nc.sync.dma_start(out=vt_sb, in_=vtd)
            nc.sync.dma_start(out=e21, in_=seld)

            nc.vector.memset(acc, 0.0)
            nc.vector.memset(ones128, 1.0)

            # ---- k column norms: rkn[m] = 1/sqrt(sum_c k[c,m]^2), broadcast
            # over 128 partitions via a ones-matmul (PE reduces partitions).
            ks2 = bigpool.tile([128, N], BF16, tag="bigb")
            ks2b = bigpool.tile([128, N], BF16, tag="bigb")
            knb = bigpool.tile([128, N], F32, tag="big")
            rkn = bigpool.tile([128, N], F32, tag="big")
            nc.vector.tensor_mul(ks2, k_sb[:, 0, :], k_sb[:, 0, :])
            nc.vector.tensor_mul(ks2b, k_sb[:, 1, :], k_sb[:, 1, :])
            for i in range(7):
                n0 = 512 * i
                cw = min(512, N - n0)
                knp = ps_n.tile([128, 512], F32, tag="psn")
                nc.tensor.matmul(
                    knp[:, :cw], ones128, ks2[:, n0 : n0 + cw], start=True, stop=False
                )
                nc.tensor.matmul(
                    knp[:, :cw], ones128, ks2b[:, n0 : n0 + cw], start=False, stop=True
                )
                nc.scalar.activation(knb[:, n0 : n0 + cw], knp[:, :cw], AF.Sqrt)
            nc.vector.reciprocal_approx_fast(rkn, knb)
            nc.vector.tensor_mul(k_sb[:, 0, :], k_sb[:, 0, :], rkn)
            nc.vector.tensor_mul(k_sb[:, 1, :], k_sb[:, 1, :], rkn)

            # ---- q row norms: 1/sqrt(sum_n q[c,n]^2), then scale q in place.
            for cc in range(2):
                for i in range(7):  # NPAD = 6*512 + 128
                    n0 = 512 * i
                    cw = min(512, NPAD - n0)
                    sq = scr.tile([128, 512], BF16, tag="sq")
                    nc.scalar.activation(
                        sq[:, :cw],
                        q_sb[:, cc, n0 : n0 + cw],
                        AF.Square,
                        accum_out=qn2p[:, cc, i : i + 1],
                    )
            nc.vector.reduce_sum(qn, qn2p, axis=mybir.AxisListType.X)
            nc.scalar.activation(qn, qn, AF.Sqrt)
            nc.vector.reciprocal(qn, qn)
            for cc in range(2):
                nc.vector.tensor_scalar_mul(
                    q_sb[:, cc, :], q_sb[:, cc, :], qn[:, cc : cc + 1]
                )

            nc.vector.reduce_sum(vtot, v_sb, axis=mybir.AxisListType.X)

            invd = persist.tile([128, N], F32, tag="invd")

            # D[m] is final once every covering row-block has accumulated:
            # after block b, columns < F(b) = 128*b + 14 are final (block b+1's
            # window starts exactly there). 1/D is produced in 512-wide prefix
            # chunks inside the loop, and each row-block's left-fill and band
            # are written as soon as the columns they need are final. Only
            # the right-fills (columns past the band) wait for the end.
            def F(b):
                return N if b >= NB - 1 else min(128 * b + 14, N)

            EMITS = [3, 7, 11, 15, 19, 23, 24]

            def emit_invd_chunk(lo, hi):
                for n0 in range(lo, hi, 512):
                    cw = min(512, hi - n0)
                    dps = ps_n.tile([128, 512], F32, tag="psn")
                    nc.tensor.matmul(
                        dps[:, :cw], e21, acc[:, n0 : n0 + cw], start=True, stop=True
                    )
                    nc.scalar.activation(
                        dps[:, :cw], dps[:, :cw], AF.Identity, bias=nconst
                    )
                    nc.vector.reciprocal_approx_fast(
                        invd[:, n0 : n0 + cw], dps[:, :cw]
                    )

            def emit_left_fill(rb):
                r0, rows, w0 = _block_geom(rb)
                if w0 > 0:
                    nc.scalar.dma_start(
                        out=attnd[r0 : r0 + rows, 0:w0], in_=invd[:rows, 0:w0]
                    )

            def emit_band(rb):
                r0, rows, w0 = _block_geom(rb)
                w1 = w0 + W
                band = bandpool.tile([128, W], F32, tag="band")
                nc.vector.tensor_mul(
                    band[:rows], a_st[:rows, rb, :], invd[:rows, w0:w1]
                )
                nc.sync.dma_start(
                    out=attnd[r0 : r0 + rows, w0:w1], in_=band[:rows]
                )

            def emit_right_fill(rb, eng):
                r0, rows, w0 = _block_geom(rb)
                w1 = w0 + W
                if w1 < N:
                    eng.dma_start(
                        out=attnd[r0 : r0 + rows, w1:N], in_=invd[:rows, w1:N]
                    )

            # earliest emission point whose F() covers each block's needs
            band_at = {}
            left_at = {}
            for rb in range(NB):
                r0, rows, w0 = _block_geom(rb)
                w1 = w0 + W
                bb = next(b for b in EMITS if F(b) >= w1)
                band_at.setdefault(bb, []).append(rb)
                if w0 > 0:
                    lb = next(b for b in EMITS if F(b) >= w0)
                    left_at.setdefault(lb, []).append(rb)

            # ---- main loop: banded energies, exp, (A-1) contraction with v,
            # incremental 1/D, and interleaved attention writes.
            # vt's column CV is 1.0 on valid rows, so c_ps row CV is the
            # column-sum of (A-1) -> D[m] = N + acc[CV, m].
            prev_f = 0
            for b in range(NB):
                r0, rows, w0 = _block_geom(b)
                g_sb = gpool.tile([128, W], BF16, tag="g")
                nc.gpsimd.dma_start(out=g_sb, in_=gsd[b])
                e_ps = ps_e.tile([128, W], F32, tag="pse")
                nc.tensor.matmul(
                    e_ps,
                    q_sb[:, 0, r0 : r0 + 128],
                    k_sb[:, 0, w0 : w0 + W],
                    start=True,
                    stop=False,
                )
                nc.tensor.matmul(
                    e_ps,
                    q_sb[:, 1, r0 : r0 + 128],
                    k_sb[:, 1, w0 : w0 + W],
                    start=False,
                    stop=True,
                )
                eg = egpool.tile([128, W], F32, tag="eg")
                nc.vector.tensor_mul(eg, e_ps, g_sb)
                nc.scalar.activation(a_st[:, b, :], eg, AF.Exp)
                am = ampool.tile([128, W], BF16, tag="am")
                nc.vector.tensor_scalar_add(am, a_st[:, b, :], -1.0)
                c_ps = ps_c.tile([CV + 1, W], F32, tag="psc")
                nc.tensor.matmul(c_ps, vt_sb[:, b, :], am, start=True, stop=True)
                nc.vector.tensor_add(
                    acc[0 : CV + 1, w0 : w0 + W], acc[0 : CV + 1, w0 : w0 + W], c_ps
                )
                if b in EMITS:
                    emit_invd_chunk(prev_f, F(b))
                    prev_f = F(b)
                for rb in left_at.get(b, []):
                    emit_left_fill(rb)
                for rb in band_at.get(b, []):
                    emit_band(rb)

            # ---- tail: right fills + out_v (need the full 1/D).
            for rb in range(NB):
                emit_right_fill(rb, nc.sync if rb % 2 == 0 else nc.scalar)
            nc.vector.tensor_scalar_add(outsb, acc[0:CV, :], vtot)
            nc.vector.tensor_mul(outsb, outsb, invd[0:CV, :])
            nc.sync.dma_start(out=outvd, in_=outsb)

    nc.compile()
    return nc


def _get_nc():
    global _NC
    if _NC is None:
        _NC = _build()
    return _NC


LAST_RESULT = None


def kernel(q, k, v, gau_kernel):
    _ensure_profiling_shims()
    import ml_dtypes

    from concourse.bass_utils import run_bass_kernel_spmd

    global LAST_RESULT

    BF = ml_dtypes.bfloat16
    q = np.asarray(q, dtype=np.float32)
    k = np.asarray(k, dtype=np.float32)
    v = np.asarray(v, dtype=np.float32)
    gau_kernel = np.asarray(gau_kernel, dtype=np.float32)

    # gau band strips, shared by all cores; pad rows of the last block are 0
    # so their exp is 1 and they contribute nothing.
    gs = np.zeros((NB, 128, W), BF)
    for b in range(NB):
        r0, rows, w0 = _block_geom(b)
        gs[b, :rows, :] = gau_kernel[r0 : r0 + rows, w0 : w0 + W].astype(BF)

    # selector: lhsT with partition CV all-ones -> matmul broadcasts acc[CV,:]
    sel = np.zeros((128, 128), np.float32)
    sel[CV, :] = 1.0

    in_maps = []
    for i in range(B):
        qb = q[i].reshape(256, N)
        qp = np.zeros((128, 2, NPAD), BF)
        qp[:, :, :N] = qb.reshape(2, 128, N).transpose(1, 0, 2).astype(BF)
        kb = k[i].reshape(256, N)
        kp = np.ascontiguousarray(kb.reshape(2, 128, N).transpose(1, 0, 2)).astype(BF)
        vb = np.ascontiguousarray(v[i].reshape(CV, N))
        vt = np.zeros((128, NB, CV + 1), BF)
        for b in range(NB):
            r0, rows, w0 = _block_geom(b)
            vt[:rows, b, :CV] = vb[:, r0 : r0 + rows].T.astype(BF)
            vt[:rows, b, CV] = 1.0
        in_maps.append(
            {"qp": qp, "kp": kp, "vp": vb, "vt": vt, "gs": gs, "sel": sel}
        )

    nc = _get_nc()
    res = run_bass_kernel_spmd(nc, in_maps, core_ids=list(range(B)))
    LAST_RESULT = res
    outs = res.results
    attn = np.stack([np.asarray(outs[i]["attn"], dtype=np.float32) for i in range(B)])
    outv = np.stack(
        [np.asarray(outs[i]["outv"], dtype=np.float32) for i in range(B)]
    ).reshape(B, CV, 56, 56)
    return outv, attn
